# revision 2
# baseline (speedup 1.0000x reference)
"""TransformerConv GNN (3 layers) on 8 Trainium2 NeuronCores.

Sharding: nodes split 3750/core (padded to 3840 = 30 tiles of 128).
Edges assigned to the core owning their dst node, grouped by 128-node
dst windows. Per layer:
  node phase: ln1 (folded into weights host-side) + q/k/v/skip
    projections; q and interleaved k|v tables written to HBM (bf16).
  kv exchange: AllGather of the per-core kv shard (bf16).
  edge phase: dma_gather of kv[src] and q[dst]; edge-attr projection via
    PE matmul accumulated with the gathered k|v in PSUM; attention
    alpha/exp/message on DVE+ACT; segment softmax accumulated per dst
    window via one-hot matmuls into PSUM (one-hot built host-side, bf16).
  FFN phase: gelu/elu residual blocks, node-local.
Output head node-local; host reassembles shards.
"""
import contextlib
import math
import numpy as np

import concourse.bass as bass
import concourse.bacc as bacc
import concourse.tile as tile
from concourse import mybir, library_config
from concourse.bass_utils import run_bass_kernel_spmd

# problem dims
N, E, F, D, H, C, ED, L = 30000, 300000, 64, 128, 8, 16, 16, 3
NCORES = 8
NL = N // NCORES          # 3750 real nodes per core
NT = 30                   # node tiles per core
NLP = NT * 128            # 3840 padded nodes per core
KVROWS = NCORES * NLP     # kv table rows (global)
P = 128
G = 8                     # edge tiles per gather batch (dma_gather fails >1024 idxs/call)
B = 4                     # edge tiles per DVE op group

fp32 = mybir.dt.float32
bf16 = mybir.dt.bfloat16
i16 = mybir.dt.int16

AF = mybir.ActivationFunctionType
OP = mybir.AluOpType
AX = mybir.AxisListType


def _bcast3(ap, reps):
    """[P, k] AP -> [P, k, reps] with 0-stride last dim."""
    return bass.AP(tensor=ap.tensor, offset=ap.offset,
                   ap=[ap.ap[0], ap.ap[1], [0, reps]])


def _bcast4(ap, reps):
    """[P, b, k] AP -> [P, b, k, reps] with 0-stride last dim."""
    return bass.AP(tensor=ap.tensor, offset=ap.offset,
                   ap=[ap.ap[0], ap.ap[1], ap.ap[2], [0, reps]])


def build(tiles_per_window, skip_bias, stage=9, elvl=4):
    """Build the Bass program. tiles_per_window: NT ints, same per core."""
    assert skip_bias, "non-zero bias path not implemented"
    tot_tiles = sum(tiles_per_window)
    tot_e = tot_tiles * 128
    nbatch = math.ceil(tot_tiles / G)

    tile_win, win_first, win_last = [], [], []
    for w, tw in enumerate(tiles_per_window):
        for i in range(tw):
            tile_win.append(w)
            win_first.append(i == 0)
            win_last.append(i == tw - 1)

    nc = bacc.Bacc("TRN2", target_bir_lowering=False, debug=False,
                   num_devices=NCORES)

    # ---------------- DRAM tensors ----------------
    x_in = nc.dram_tensor("x_shard", [NLP, F], fp32, kind="ExternalInput").ap()
    idx_src_d = nc.dram_tensor("idx_src", [P, tot_e // 16], i16,
                               kind="ExternalInput").ap()
    idx_dst_d = nc.dram_tensor("idx_dst", [P, tot_e // 16], i16,
                               kind="ExternalInput").ap()
    oh_d = nc.dram_tensor("onehot", [tot_e, P], bf16, kind="ExternalInput").ap()
    ea_d = nc.dram_tensor("ea_t", [ED, tot_e], bf16, kind="ExternalInput").ap()
    wq_d = nc.dram_tensor("wqT", [L, D, D], fp32, kind="ExternalInput").ap()
    wk_d = nc.dram_tensor("wkT", [L, D, D], fp32, kind="ExternalInput").ap()
    wv_d = nc.dram_tensor("wvT", [L, D, D], fp32, kind="ExternalInput").ap()
    ws_d = nc.dram_tensor("wsT", [L, D, D], fp32, kind="ExternalInput").ap()
    w1_d = nc.dram_tensor("w1T", [L, D, D], fp32, kind="ExternalInput").ap()
    w2_d = nc.dram_tensor("w2T", [L, D, D], fp32, kind="ExternalInput").ap()
    ewd_d = nc.dram_tensor("ewdT", [L, ED, 2 * D], bf16,
                           kind="ExternalInput").ap()
    w0_d = nc.dram_tensor("w0T", [F, D], fp32, kind="ExternalInput").ap()
    id_d = nc.dram_tensor("ident", [P, P], fp32, kind="ExternalInput").ap()
    wl_d = nc.dram_tensor("wlT", [D, 4], fp32, kind="ExternalInput").ap()
    out_d = nc.dram_tensor("out", [NLP, 4], fp32, kind="ExternalOutput").ap()

    q_tab = nc.dram_tensor("q_tab", [NLP, D], bf16).ap()
    kv_bounce = nc.dram_tensor("kv_bounce", [NLP, 2 * D], bf16).ap()
    kv_full = nc.dram_tensor("kv_full", [KVROWS, 2 * D], bf16,
                             addr_space="Shared").ap()

    eps = 1e-5

    with tile.TileContext(nc) as tc:
        nc.gpsimd.load_library(library_config.mlp)
        with contextlib.ExitStack() as ctx:
            const = ctx.enter_context(tc.tile_pool(name="const", bufs=1))
            nodes = ctx.enter_context(tc.tile_pool(name="nodes", bufs=1))
            wpool = ctx.enter_context(tc.tile_pool(name="wpool", bufs=2))
            ntmp = ctx.enter_context(tc.tile_pool(name="ntmp", bufs=3))
            nsm = ctx.enter_context(tc.tile_pool(name="nsm", bufs=4))
            gbuf = ctx.enter_context(tc.tile_pool(name="gbuf", bufs=2))
            ebuf = ctx.enter_context(tc.tile_pool(name="ebuf", bufs=3))

            # constants
            id32 = const.tile([P, P], fp32, tag="id32")
            nc.sync.dma_start(out=id32[:], in_=id_d[:, :])
            id16 = const.tile([P, P], bf16, tag="id16")
            nc.vector.tensor_copy(out=id16[:], in_=id32[:])
            eps_t = const.tile([P, 1], fp32, tag="eps")
            nc.vector.memset(eps_t[:], eps)

            idx_src = const.tile([P, tot_e // 16], i16, tag="isrc")
            nc.sync.dma_start(out=idx_src[:], in_=idx_src_d[:, :])
            idx_dst = const.tile([P, tot_e // 16], i16, tag="idst")
            nc.sync.dma_start(out=idx_dst[:], in_=idx_dst_d[:, :])

            h_t = nodes.tile([P, NT, D], fp32, tag="h")
            skip_t = nodes.tile([P, NT, D], fp32, tag="skip")
            hc_t = nodes.tile([P, NT, D], fp32, tag="hc")

            def layernorm_tile(x_ap, out_ap):
                st = nsm.tile([P, 6], fp32, tag="st", name="st")
                nc.vector.bn_stats(out=st[:], in_=x_ap)
                mv = nsm.tile([P, 2], fp32, tag="mv", name="mv")
                nc.vector.bn_aggr(out=mv[:], in_=st[:])
                sd = nsm.tile([P, 1], fp32, tag="sd", name="sd")
                nc.scalar.activation(out=sd[:], in_=mv[:, 1:2], func=AF.Sqrt,
                                     bias=eps_t[:], scale=1.0)
                rs = nsm.tile([P, 1], fp32, tag="rs", name="rs")
                nc.vector.reciprocal(out=rs[:], in_=sd[:])
                nc.vector.scalar_tensor_tensor(
                    out=out_ap, in0=x_ap, scalar=mv[:, 0:1],
                    in1=rs[:].to_broadcast([P, D]),
                    op0=OP.subtract, op1=OP.mult)

            def transpose_to(x_ap, psum_pool):
                tp = psum_pool.tile([P, P], fp32, space="PSUM", tag="tp",
                                    name="tp")
                nc.tensor.transpose(out=tp[:], in_=x_ap, identity=id32[:])
                ts = ntmp.tile([P, P], fp32, tag="tT", name="ts")
                nc.scalar.copy(out=ts[:], in_=tp[:])
                return ts

            def elu_from_psum(ps_ap, out_ap):
                mn = nsm.tile([P, D], fp32, tag="mn", name="mn")
                nc.vector.tensor_scalar_min(mn[:], ps_ap, 0.0)
                em = nsm.tile([P, D], fp32, tag="em", name="em")
                nc.scalar.activation(out=em[:], in_=mn[:], func=AF.Exp)
                mx = nsm.tile([P, D], fp32, tag="mx", name="mx")
                nc.vector.tensor_scalar_max(mx[:], ps_ap, 0.0)
                nc.vector.scalar_tensor_tensor(
                    out=out_ap, in0=em[:], scalar=-1.0, in1=mx[:],
                    op0=OP.add, op1=OP.add)

            # ---------------- input projection ----------------
            w0 = const.tile([F, D], fp32, tag="w0")
            nc.sync.dma_start(out=w0[:], in_=w0_d[:, :])
            with tc.tile_pool(name="ps0", bufs=2, space="PSUM") as ps0:
                for t in range(NT):
                    xt = ntmp.tile([P, F], fp32, tag="xt", name="xt")
                    nc.sync.dma_start(out=xt[:],
                                      in_=x_in[t * P:(t + 1) * P, :])
                    tp = ps0.tile([P, P], fp32, space="PSUM", tag="tp",
                                  name="tp")
                    nc.tensor.transpose(out=tp[:F, :], in_=xt[:],
                                        identity=id32[:])
                    xT = ntmp.tile([F, P], fp32, tag="tT", name="xT")
                    nc.scalar.copy(out=xT[:], in_=tp[:F, :])
                    h0 = ps0.tile([P, D], fp32, space="PSUM", tag="mm",
                                  name="h0")
                    nc.tensor.matmul(out=h0[:], lhsT=xT[:], rhs=w0[:],
                                     start=True, stop=True)
                    elu_from_psum(h0[:], h_t[:, t, :])

            # ---------------- layers ----------------
            for l in range(L if stage >= 9 else (1 if stage >= 1 else 0)):
                wq = wpool.tile([D, D], fp32, tag="wq", name="wq")
                nc.sync.dma_start(out=wq[:], in_=wq_d[l])
                wk = wpool.tile([D, D], fp32, tag="wk", name="wk")
                nc.sync.dma_start(out=wk[:], in_=wk_d[l])
                wv = wpool.tile([D, D], fp32, tag="wv", name="wv")
                nc.sync.dma_start(out=wv[:], in_=wv_d[l])
                ws = wpool.tile([D, D], fp32, tag="ws", name="ws")
                nc.sync.dma_start(out=ws[:], in_=ws_d[l])
                w1 = wpool.tile([D, D], fp32, tag="w1", name="w1")
                nc.sync.dma_start(out=w1[:], in_=w1_d[l])
                w2 = wpool.tile([D, D], fp32, tag="w2", name="w2")
                nc.sync.dma_start(out=w2[:], in_=w2_d[l])
                ewd = wpool.tile([ED, 2 * D], bf16, tag="ewd", name="ewd")
                nc.sync.dma_start(out=ewd[:], in_=ewd_d[l])

                # ---- node phase ----
                with tc.tile_pool(name=f"npsA{l}", bufs=2, space="PSUM") as nps:
                    for t in range(NT):
                        hn = ntmp.tile([P, D], fp32, tag="hn", name="hn")
                        layernorm_tile(h_t[:, t, :], hn[:])
                        hnT = transpose_to(hn[:], nps)
                        qp = nps.tile([P, D], fp32, space="PSUM", tag="mm",
                                      bufs=4, name="qp")
                        nc.tensor.matmul(out=qp[:], lhsT=hnT[:], rhs=wq[:],
                                         start=True, stop=True)
                        qb = ntmp.tile([P, D], bf16, tag="qb", name="qb")
                        nc.scalar.copy(out=qb[:], in_=qp[:])
                        nc.sync.dma_start(out=q_tab[t * P:(t + 1) * P, :],
                                          in_=qb[:])
                        kp = nps.tile([P, D], fp32, space="PSUM", tag="mm",
                                      bufs=4, name="kp")
                        nc.tensor.matmul(out=kp[:], lhsT=hnT[:], rhs=wk[:],
                                         start=True, stop=True)
                        vp = nps.tile([P, D], fp32, space="PSUM", tag="mm",
                                      bufs=4, name="vp")
                        nc.tensor.matmul(out=vp[:], lhsT=hnT[:], rhs=wv[:],
                                         start=True, stop=True)
                        kvb = ntmp.tile([P, 2 * D], bf16, tag="kvb",
                                        name="kvb")
                        nc.scalar.copy(out=kvb[:, :D], in_=kp[:])
                        nc.scalar.copy(out=kvb[:, D:], in_=vp[:])
                        nc.sync.dma_start(out=kv_bounce[t * P:(t + 1) * P, :],
                                          in_=kvb[:])
                        sp = nps.tile([P, D], fp32, space="PSUM", tag="mm",
                                      bufs=4, name="sp")
                        nc.tensor.matmul(out=sp[:], lhsT=hnT[:], rhs=ws[:],
                                         start=True, stop=True)
                        nc.vector.tensor_copy(out=skip_t[:, t, :], in_=sp[:])

                if stage < 9 and stage < 1:
                    break
                # ---- kv exchange ----
                nc.gpsimd.collective_compute(
                    "AllGather", OP.bypass,
                    replica_groups=[list(range(NCORES))],
                    ins=[kv_bounce.opt()], outs=[kv_full.opt()])

                # ---- edge phase ----
                if stage < 9 and stage < 2:
                    continue
                with tc.tile_pool(name=f"epsK{l}", bufs=2, space="PSUM") \
                        as eps_ps, \
                        tc.tile_pool(name=f"epsA{l}", bufs=2, space="PSUM") \
                        as acc_ps:
                    acc_tiles = {}
                    for g in range(nbatch):
                        t0 = g * G
                        gb = min(G, tot_tiles - t0)
                        ne = gb * 128
                        kvg = gbuf.tile([P, G, 2 * D], bf16, tag="kvg",
                                        name="kvg")
                        nc.gpsimd.dma_gather(
                            kvg[:, :gb, :], kv_full[:],
                            idx_src[:, t0 * 8:t0 * 8 + ne // 16],
                            ne, ne, 2 * D)
                        qg = gbuf.tile([P, G, D], bf16, tag="qg", name="qg")
                        nc.gpsimd.dma_gather(
                            qg[:, :gb, :], q_tab[:],
                            idx_dst[:, t0 * 8:t0 * 8 + ne // 16],
                            ne, ne, D)
                        oh = gbuf.tile([P, G, P], bf16, tag="oh", name="oh")
                        nc.sync.dma_start(
                            out=oh[:, :gb, :],
                            in_=oh_d.rearrange("(b p) n -> p b n", p=P)[
                                :, t0:t0 + gb, :])
                        eat = gbuf.tile([ED, G * 128], bf16, tag="eat",
                                        name="eat")
                        nc.sync.dma_start(
                            out=eat[:, :ne],
                            in_=ea_d[:, t0 * 128:t0 * 128 + ne])

                        if elvl < 2:
                            continue
                        for bb in range(math.ceil(gb / B)):
                            nb = min(B, gb - bb * B)
                            kvpe = eps_ps.tile([P, B, 2 * D], fp32,
                                               space="PSUM", tag="kvpe",
                                               name="kvpe")
                            for u in range(nb):
                                te = bb * B + u
                                nc.tensor.matmul(
                                    out=kvpe[:, u, :],
                                    lhsT=eat[:, te * 128:(te + 1) * 128],
                                    rhs=ewd[:], start=True, stop=False,
                                    skip_group_check=True)
                                nc.tensor.matmul(
                                    out=kvpe[:, u, :], lhsT=id16[:],
                                    rhs=kvg[:, te, :], start=False, stop=True,
                                    skip_group_check=True)
                            if elvl < 3:
                                continue
                            qs = bb * B
                            qk = ebuf.tile([P, B, D], fp32, tag="qk",
                                           name="qk")
                            nc.vector.tensor_tensor(
                                out=qk[:, :nb, :].rearrange(
                                    "p b (h c) -> p b h c", h=H),
                                in0=qg[:, qs:qs + nb, :].rearrange(
                                    "p b (h c) -> p b h c", h=H),
                                in1=kvpe[:, :nb, :D].rearrange(
                                    "p b (h c) -> p b h c", h=H),
                                op=OP.mult)
                            al = ebuf.tile([P, B, H], fp32, tag="al",
                                           name="al")
                            nc.vector.tensor_reduce(
                                out=al[:, :nb, :],
                                in_=qk[:, :nb, :].rearrange(
                                    "p b (h c) -> p b h c", h=H),
                                axis=AX.X, op=OP.add)
                            pk = ebuf.tile([P, B, D + 8], bf16, tag="pk",
                                           name="pk")
                            nc.scalar.activation(
                                out=pk[:, :nb, D:], in_=al[:, :nb, :],
                                func=AF.Exp, scale=1.0 / math.sqrt(C))
                            nc.vector.tensor_tensor(
                                out=pk[:, :nb, :D].rearrange(
                                    "p b (h c) -> p b h c", h=H),
                                in0=kvpe[:, :nb, D:].rearrange(
                                    "p b (h c) -> p b h c", h=H),
                                in1=_bcast4(pk[:, :nb, D:], C),
                                op=OP.mult)
                            if elvl < 4:
                                continue
                            for u in range(nb):
                                tid = t0 + bb * B + u
                                w = tile_win[tid]
                                if win_first[tid]:
                                    acc_tiles[w] = acc_ps.tile(
                                        [P, D + 8], fp32, space="PSUM",
                                        tag="acc", name="acc")
                                nc.tensor.matmul(
                                    out=acc_tiles[w][:],
                                    lhsT=oh[:, bb * B + u, :],
                                    rhs=pk[:, u, :],
                                    start=win_first[tid], stop=win_last[tid],
                                    skip_group_check=True)
                                if win_last[tid]:
                                    ac = acc_tiles.pop(w)
                                    dn = nsm.tile([P, H], fp32, tag="dn",
                                                  name="dn")
                                    nc.vector.tensor_scalar_add(
                                        dn[:], ac[:, D:], 1e-16)
                                    rd = nsm.tile([P, H], fp32, tag="rd",
                                                  name="rd")
                                    nc.vector.reciprocal(out=rd[:], in_=dn[:])
                                    mg = ntmp.tile([P, D], fp32, tag="mg",
                                                   name="mg")
                                    nc.vector.tensor_tensor(
                                        out=mg[:].rearrange(
                                            "p (h c) -> p h c", h=H),
                                        in0=ac[:, :D].rearrange(
                                            "p (h c) -> p h c", h=H),
                                        in1=_bcast3(rd[:], C), op=OP.mult)
                                    nc.vector.tensor_tensor(
                                        out=hc_t[:, w, :], in0=mg[:],
                                        in1=skip_t[:, w, :], op=OP.add)

                # ---- FFN phase ----
                if stage < 9 and stage < 3:
                    continue
                with tc.tile_pool(name=f"npsB{l}", bufs=2, space="PSUM") \
                        as fps:
                    for t in range(NT):
                        hcT = transpose_to(hc_t[:, t, :], fps)
                        t1p = fps.tile([P, D], fp32, space="PSUM", tag="mm",
                                       name="t1p")
                        nc.tensor.matmul(out=t1p[:], lhsT=hcT[:], rhs=w1[:],
                                         start=True, stop=True)
                        t1g = ntmp.tile([P, D], fp32, tag="t1g", name="t1g")
                        nc.scalar.activation(out=t1g[:], in_=t1p[:],
                                             func=AF.Gelu)
                        idn = ntmp.tile([P, D], fp32, tag="idn", name="idn")
                        nc.vector.tensor_tensor(out=idn[:], in0=t1g[:],
                                                in1=h_t[:, t, :], op=OP.add)
                        t2 = ntmp.tile([P, D], fp32, tag="hn", name="t2")
                        layernorm_tile(idn[:], t2[:])
                        t2T = transpose_to(t2[:], fps)
                        t3p = fps.tile([P, D], fp32, space="PSUM", tag="mm",
                                       name="t3p")
                        nc.tensor.matmul(out=t3p[:], lhsT=t2T[:], rhs=w2[:],
                                         start=True, stop=True)
                        t4 = ntmp.tile([P, D], fp32, tag="t4", name="t4")
                        elu_from_psum(t3p[:], t4[:])
                        nc.vector.tensor_tensor(out=h_t[:, t, :], in0=t4[:],
                                                in1=idn[:], op=OP.add)

            # ---------------- output head ----------------
            wl = const.tile([D, 4], fp32, tag="wl")
            nc.sync.dma_start(out=wl[:], in_=wl_d[:, :])
            with tc.tile_pool(name="psH", bufs=2, space="PSUM") as psh:
                for t in range(NT):
                    hn = ntmp.tile([P, D], fp32, tag="hn", name="hnl")
                    layernorm_tile(h_t[:, t, :], hn[:])
                    hnT = transpose_to(hn[:], psh)
                    op_ = psh.tile([P, 4], fp32, space="PSUM", tag="mm",
                                   name="op")
                    nc.tensor.matmul(out=op_[:], lhsT=hnT[:], rhs=wl[:],
                                     start=True, stop=True)
                    ot = ntmp.tile([P, 4], fp32, tag="ot", name="ot")
                    nc.scalar.copy(out=ot[:], in_=op_[:])
                    nc.sync.dma_start(out=out_d[t * P:(t + 1) * P, :],
                                      in_=ot[:])

    nc.compile()
    return nc


def prep_inputs(x, edge_index, edge_attr,
                lin0_w, lin0_b,
                q_w, q_b, k_w, k_b, v_w, v_b, e_w, skip_w, skip_b,
                ln1_g, ln1_b, lins_w, lins_b, ln2_g, ln2_b,
                lins2_w, lins2_b, lnl_g, lnl_b, linl_w, linl_b):
    """Host-side sharding/sorting/folding."""
    x = np.asarray(x, np.float32)
    ei = np.asarray(edge_index, np.int64)
    ea = np.asarray(edge_attr, np.float32)
    src, dst = ei[0], ei[1]
    core = dst // NL
    slot = dst - core * NL

    def fold(W, bias, g, b):
        W = np.asarray(W, np.float64)
        Wf = W * np.asarray(g, np.float64)[None, :]
        cf = np.asarray(bias, np.float64) + W @ np.asarray(b, np.float64)
        return Wf.astype(np.float32), cf.astype(np.float32)

    wqT = np.zeros((L, D, D), np.float32)
    wkT = np.zeros((L, D, D), np.float32)
    wvT = np.zeros((L, D, D), np.float32)
    wsT = np.zeros((L, D, D), np.float32)
    w1T = np.zeros((L, D, D), np.float32)
    w2T = np.zeros((L, D, D), np.float32)
    ewdT = np.zeros((L, ED, 2 * D), np.float32)
    zero_bias = True
    for l in range(L):
        for (W, bias, dstT) in [(q_w[l], q_b[l], wqT), (k_w[l], k_b[l], wkT),
                                (v_w[l], v_b[l], wvT),
                                (skip_w[l], skip_b[l], wsT)]:
            Wf, cf = fold(W, bias, ln1_g[l], ln1_b[l])
            dstT[l] = Wf.T
            zero_bias &= bool(np.abs(cf).max() == 0)
        w1T[l] = np.asarray(lins_w[l]).T
        zero_bias &= bool(np.abs(np.asarray(lins_b[l])).max() == 0)
        Wf, cf = fold(lins2_w[l], lins2_b[l], ln2_g[l], ln2_b[l])
        w2T[l] = Wf.T
        zero_bias &= bool(np.abs(cf).max() == 0)
        ewT = np.asarray(e_w[l]).T.astype(np.float32)   # [ED, D]
        ewdT[l, :, :D] = ewT
        ewdT[l, :, D:] = ewT
    Wl, cl = fold(linl_w, linl_b, lnl_g, lnl_b)
    wlT = np.zeros((D, 4), np.float32)
    wlT[:, :3] = Wl.T
    zero_bias &= bool(np.abs(cl).max() == 0)
    zero_bias &= bool(np.abs(np.asarray(lin0_b)).max() == 0)

    win = slot // 128
    counts = np.zeros((NCORES, NT), np.int64)
    np.add.at(counts, (core, win), 1)
    tiles_per_window = [max(1, int(math.ceil(counts[:, w].max() / 128)))
                        for w in range(NT)]
    tot_tiles = sum(tiles_per_window)
    tot_e = tot_tiles * 128

    in_maps = []
    order_all = np.lexsort((win, core))
    off = np.searchsorted(core[order_all], np.arange(NCORES + 1))
    kvrow_of = (src // NL) * NLP + (src % NL)

    for c in range(NCORES):
        oc = order_all[off[c]:off[c + 1]]
        wc = win[oc]
        woff = np.searchsorted(wc, np.arange(NT + 1))
        src_rows = np.zeros(tot_e, np.int16)
        dst_rows = np.zeros(tot_e, np.int16)
        onehot = np.zeros((tot_e, P), np.float32)
        ea_t = np.zeros((ED, tot_e), np.float32)
        base = 0
        for w in range(NT):
            ew_idx = oc[woff[w]:woff[w + 1]]
            k = len(ew_idx)
            sl = slice(base, base + k)
            src_rows[sl] = kvrow_of[ew_idx].astype(np.int16)
            dst_rows[sl] = slot[ew_idx].astype(np.int16)
            onehot[np.arange(base, base + k), slot[ew_idx] - w * 128] = 1.0
            ea_t[:, sl] = ea[ew_idx].T
            base += tiles_per_window[w] * 128
        assert base == tot_e

        def wrap(a):
            return np.tile(a.reshape(tot_e // 16, 16).T, (8, 1)).copy()

        xs = np.zeros((NLP, F), np.float32)
        xs[:NL] = x[c * NL:(c + 1) * NL]
        in_maps.append({
            "x_shard": xs,
            "idx_src": wrap(src_rows),
            "idx_dst": wrap(dst_rows),
            "onehot": onehot,
            "ea_t": ea_t,
            "wqT": wqT, "wkT": wkT, "wvT": wvT, "wsT": wsT,
            "w1T": w1T, "w2T": w2T, "ewdT": ewdT,
            "w0T": np.asarray(lin0_w).T.astype(np.float32),
            "ident": np.eye(P, dtype=np.float32),
            "wlT": wlT,
        })
    return in_maps, tiles_per_window, zero_bias


_CACHE = {}


def kernel(**inputs):
    import ml_dtypes
    in_maps, tiles_per_window, zero_bias = prep_inputs(**inputs)
    for m in in_maps:
        m["onehot"] = m["onehot"].astype(ml_dtypes.bfloat16)
        m["ea_t"] = m["ea_t"].astype(ml_dtypes.bfloat16)
        m["ewdT"] = m["ewdT"].astype(ml_dtypes.bfloat16)

    import os
    stage = int(os.environ.get("K_STAGE", "9"))
    elvl = int(os.environ.get("K_EDGE", "4"))
    key = (tuple(tiles_per_window), stage, elvl)
    if key not in _CACHE:
        _CACHE[key] = build(tiles_per_window, zero_bias, stage, elvl)
    nc = _CACHE[key]

    global LAST_RESULT
    res = run_bass_kernel_spmd(
        nc, in_maps, core_ids=list(range(NCORES)),
        trace=bool(os.environ.get("K_TRACE")))
    LAST_RESULT = res
    out = np.zeros((N, 3), np.float32)
    for c in range(NCORES):
        out[c * NL:(c + 1) * NL] = res.results[c]["out"][:NL, :3]
    return out



# revision 12
# speedup vs baseline: 1.6538x; 1.6538x over previous
"""TransformerConv GNN (3 layers) on 8 Trainium2 NeuronCores.

Sharding: nodes split 3750/core (padded to 3840 = 30 tiles of 128).
Edges assigned to the core owning their dst node, grouped by 128-node
dst windows. Per layer:
  node phase: ln1 (folded into weights host-side); fused q|k|v|skip
    projection as ONE bf16 matmul per tile ([D, 4D] moving operand);
    interleaved k|v table written to HBM (bf16).
  kv exchange: AllGather of the per-core kv shard (bf16).
  edge phase: dma_gather of kv[src] only (q never leaves SBUF: per-edge
    q comes from a PE matmul with the transposed dst one-hot as the
    stationary operand, resident in SBUF across layers). The scatter
    one-hot is generated on-chip per tile (DVE is_equal vs an iota row).
    Edge-attr projection accumulated with gathered k|v in PSUM; alpha/
    exp/message on DVE+ACT; segment softmax accumulated per dst window
    via one-hot matmuls into PSUM.
  FFN phase: gelu/elu residual blocks, node-local, batched op-major so
    the ACT engine's function table rarely reloads.
Output head node-local; host reassembles shards.
"""
import contextlib
import math
import os
import numpy as np

import concourse.bass as bass
import concourse.bacc as bacc
import concourse.tile as tile
from concourse import mybir, library_config
from concourse.bass_utils import run_bass_kernel_spmd

# problem dims
N, E, F, D, H, C, ED, L = 30000, 300000, 64, 128, 8, 16, 16, 3
NCORES = 8
NL = N // NCORES          # 3750 real nodes per core
NT = 30                   # node tiles per core
NLP = NT * 128            # 3840 padded nodes per core
KVROWS = NCORES * NLP     # kv table rows (global)
P = 128
G = 8                     # edge tiles per gather batch (dma_gather <=1024 idxs)
B = 4                     # edge tiles per DVE op group

fp32 = mybir.dt.float32
bf16 = mybir.dt.bfloat16
i16 = mybir.dt.int16

AF = mybir.ActivationFunctionType
OP = mybir.AluOpType
AX = mybir.AxisListType

LAST_RESULT = None


def _bcast3(ap, reps):
    """[P, k] AP -> [P, k, reps] with 0-stride last dim."""
    return bass.AP(tensor=ap.tensor, offset=ap.offset,
                   ap=[ap.ap[0], ap.ap[1], [0, reps]])


def _bcast4(ap, reps):
    """[P, b, k] AP -> [P, b, k, reps] with 0-stride last dim."""
    return bass.AP(tensor=ap.tensor, offset=ap.offset,
                   ap=[ap.ap[0], ap.ap[1], ap.ap[2], [0, reps]])


def _bcast_mid(ap, reps):
    """[P, k] AP -> [P, reps, k] with 0-stride middle dim."""
    return bass.AP(tensor=ap.tensor, offset=ap.offset,
                   ap=[ap.ap[0], [0, reps], ap.ap[1]])


def build(tiles_per_window, skip_bias, stage=9, elvl=4):
    """Build the Bass program. tiles_per_window: NT ints, same per core."""
    assert skip_bias, "non-zero bias path not implemented"
    tot_tiles = sum(tiles_per_window)
    tot_e = tot_tiles * 128
    nbatch = math.ceil(tot_tiles / G)

    tile_win, win_first, win_last = [], [], []
    for w, tw in enumerate(tiles_per_window):
        for i in range(tw):
            tile_win.append(w)
            win_first.append(i == 0)
            win_last.append(i == tw - 1)

    nc = bacc.Bacc("TRN2", target_bir_lowering=False, debug=False,
                   num_devices=NCORES)

    # ---------------- DRAM tensors ----------------
    x_in = nc.dram_tensor("x_shard", [NLP, F], fp32, kind="ExternalInput").ap()
    idx_src_d = nc.dram_tensor("idx_src", [P, tot_e // 16], i16,
                               kind="ExternalInput").ap()
    ohT_d = nc.dram_tensor("ohT", [P, tot_e], bf16, kind="ExternalInput").ap()
    dcol_d = nc.dram_tensor("dst_col", [P, tot_tiles], fp32,
                            kind="ExternalInput").ap()
    rix_d = nc.dram_tensor("rowidx", [P, P], fp32, kind="ExternalInput").ap()
    ea_d = nc.dram_tensor("ea_t", [ED, tot_e], bf16, kind="ExternalInput").ap()
    w4_d = nc.dram_tensor("w4T", [L, D, 4 * D], bf16,
                          kind="ExternalInput").ap()
    w1_d = nc.dram_tensor("w1T", [L, D, D], bf16, kind="ExternalInput").ap()
    w2_d = nc.dram_tensor("w2T", [L, D, D], bf16, kind="ExternalInput").ap()
    ewd_d = nc.dram_tensor("ewdT", [L, ED, 2 * D], bf16,
                           kind="ExternalInput").ap()
    w0_d = nc.dram_tensor("w0T", [F, D], fp32, kind="ExternalInput").ap()
    id_d = nc.dram_tensor("ident", [P, P], fp32, kind="ExternalInput").ap()
    wl_d = nc.dram_tensor("wlT", [D, 4], fp32, kind="ExternalInput").ap()
    out_d = nc.dram_tensor("out", [NLP, 4], fp32, kind="ExternalOutput").ap()

    kv_bounce = nc.dram_tensor("kv_bounce", [NLP, 2 * D], bf16).ap()
    kv_full = nc.dram_tensor("kv_full", [KVROWS, 2 * D], bf16,
                             addr_space="Shared").ap()

    eps = 1e-5

    with tile.TileContext(nc) as tc:
        nc.gpsimd.load_library(library_config.mlp)
        with contextlib.ExitStack() as ctx:
            const = ctx.enter_context(tc.tile_pool(name="const", bufs=1))
            nodes = ctx.enter_context(tc.tile_pool(name="nodes", bufs=1))
            wpool = ctx.enter_context(tc.tile_pool(name="wpool", bufs=2))
            ntmp = ctx.enter_context(tc.tile_pool(name="ntmp", bufs=3))
            nsm = ctx.enter_context(tc.tile_pool(name="nsm", bufs=4))
            gbuf = ctx.enter_context(tc.tile_pool(name="gbuf", bufs=2))
            ebuf = ctx.enter_context(tc.tile_pool(name="ebuf", bufs=3))
            nbig = ctx.enter_context(tc.tile_pool(name="nbig", bufs=2))

            # constants
            id32 = const.tile([P, P], fp32, tag="id32")
            nc.sync.dma_start(out=id32[:], in_=id_d[:, :])
            id16 = const.tile([P, P], bf16, tag="id16")
            nc.vector.tensor_copy(out=id16[:], in_=id32[:])
            eps_t = const.tile([P, 1], fp32, tag="eps")
            nc.vector.memset(eps_t[:], eps)
            rix = const.tile([P, P], fp32, tag="rix")
            nc.sync.dma_start(out=rix[:], in_=rix_d[:, :])
            dcol = const.tile([P, tot_tiles], fp32, tag="dcol")
            nc.sync.dma_start(out=dcol[:], in_=dcol_d[:, :])

            idx_src = const.tile([P, tot_e // 16], i16, tag="isrc")
            nc.sync.dma_start(out=idx_src[:], in_=idx_src_d[:, :])

            h_t = nodes.tile([P, NT, D], fp32, tag="h")
            skip_t = nodes.tile([P, NT, D], fp32, tag="skip")
            hc_t = nodes.tile([P, NT, D], fp32, tag="hc")
            idn_t = nodes.tile([P, NT, D], fp32, tag="idn")
            q_sb = nodes.tile([P, NT, D], bf16, tag="qsb")

            def ln_batch(src_t, dst_of):
                """Batched layernorm of src_t[:, t, :] for all NT tiles.
                dst_of(t) -> output AP for tile t."""
                sts = nbig.tile([P, NT, 6], fp32, tag="sts", name="sts")
                mvs = nsm.tile([P, NT, 2], fp32, tag="mvs", name="mvs")
                for t in range(NT):
                    nc.vector.bn_stats(out=sts[:, t, :], in_=src_t[:, t, :])
                    nc.vector.bn_aggr(out=mvs[:, t, :], in_=sts[:, t, :])
                sd = nsm.tile([P, NT], fp32, tag="sd", name="sd")
                nc.scalar.activation(out=sd[:], in_=mvs[:, :, 1:2],
                                     func=AF.Sqrt, bias=eps_t[:], scale=1.0)
                rs = nsm.tile([P, NT], fp32, tag="rs", name="rs")
                nc.vector.reciprocal(out=rs[:], in_=sd[:])
                for t in range(NT):
                    nc.vector.scalar_tensor_tensor(
                        out=dst_of(t), in0=src_t[:, t, :],
                        scalar=mvs[:, t, 0:1],
                        in1=rs[:, t:t + 1].to_broadcast([P, D]),
                        op0=OP.subtract, op1=OP.mult)

            # ---------------- input projection ----------------
            w0 = const.tile([F, D], fp32, tag="w0")
            nc.sync.dma_start(out=w0[:], in_=w0_d[:, :])
            with tc.tile_pool(name="ps0", bufs=2, space="PSUM") as ps0:
                for t in range(NT):
                    xt = ntmp.tile([P, F], fp32, tag="xt", name="xt")
                    nc.sync.dma_start(out=xt[:],
                                      in_=x_in[t * P:(t + 1) * P, :])
                    tp = ps0.tile([P, P], fp32, space="PSUM", tag="tp",
                                  name="tp")
                    nc.tensor.transpose(out=tp[:F, :], in_=xt[:],
                                        identity=id32[:])
                    xT = ntmp.tile([F, P], fp32, tag="tT", name="xT")
                    nc.vector.tensor_copy(out=xT[:], in_=tp[:F, :])
                    h0 = ps0.tile([P, D], fp32, space="PSUM", tag="mm",
                                  name="h0")
                    nc.tensor.matmul(out=h0[:], lhsT=xT[:], rhs=w0[:],
                                     start=True, stop=True)
                    # elu from psum (exp on ACT, rest on DVE)
                    mn = nsm.tile([P, D], fp32, tag="mn", name="mn")
                    nc.vector.tensor_scalar_min(mn[:], h0[:], 0.0)
                    em = nsm.tile([P, D], fp32, tag="em", name="em")
                    nc.scalar.activation(out=em[:], in_=mn[:], func=AF.Exp)
                    mx = nsm.tile([P, D], fp32, tag="mx", name="mx")
                    nc.vector.tensor_scalar_max(mx[:], h0[:], 0.0)
                    nc.vector.scalar_tensor_tensor(
                        out=h_t[:, t, :], in0=em[:], scalar=-1.0, in1=mx[:],
                        op0=OP.add, op1=OP.add)

            # ---------------- layers ----------------
            for l in range(L if stage >= 9 else (1 if stage >= 1 else 0)):
                w4 = wpool.tile([D, 4 * D], bf16, tag="w4", name="w4")
                nc.sync.dma_start(out=w4[:], in_=w4_d[l])
                w1 = wpool.tile([D, D], bf16, tag="w1", name="w1")
                nc.sync.dma_start(out=w1[:], in_=w1_d[l])
                w2 = wpool.tile([D, D], bf16, tag="w2", name="w2")
                nc.sync.dma_start(out=w2[:], in_=w2_d[l])
                ewd = wpool.tile([ED, 2 * D], bf16, tag="ewd", name="ewd")
                nc.sync.dma_start(out=ewd[:], in_=ewd_d[l])

                # ---- node phase ----
                hn_t = nbig.tile([P, NT, D], fp32, tag="hn_t", name="hn_t")
                ln_batch(h_t, lambda t: hn_t[:, t, :])
                with tc.tile_pool(name=f"npsA{l}", bufs=2, space="PSUM") \
                        as nps:
                    for t in range(NT):
                        tp = nps.tile([P, P], fp32, space="PSUM", tag="tp",
                                      name="tp")
                        nc.tensor.transpose(out=tp[:], in_=hn_t[:, t, :],
                                            identity=id32[:])
                        hnT = ntmp.tile([P, P], bf16, tag="hnT", name="hnT")
                        nc.scalar.copy(out=hnT[:], in_=tp[:])
                        p4 = nps.tile([P, 4 * D], fp32, space="PSUM",
                                      tag="mm", name="p4")
                        nc.tensor.matmul(out=p4[:], lhsT=hnT[:], rhs=w4[:],
                                         start=True, stop=True)
                        nc.scalar.copy(out=q_sb[:, t, :], in_=p4[:, :D])
                        kvb = ntmp.tile([P, 2 * D], bf16, tag="kvb",
                                        name="kvb")
                        nc.scalar.copy(out=kvb[:], in_=p4[:, D:3 * D])
                        nc.sync.dma_start(out=kv_bounce[t * P:(t + 1) * P, :],
                                          in_=kvb[:])
                        nc.vector.tensor_copy(out=skip_t[:, t, :],
                                              in_=p4[:, 3 * D:])

                if stage < 9 and stage < 1:
                    break
                # ---- kv exchange ----
                nc.gpsimd.collective_compute(
                    "AllGather", OP.bypass,
                    replica_groups=[list(range(NCORES))],
                    ins=[kv_bounce.opt()], outs=[kv_full.opt()])

                # ---- edge phase ----
                if stage < 9 and stage < 2:
                    continue
                with tc.tile_pool(name=f"epsK{l}", bufs=2, space="PSUM") \
                        as eps_ps, \
                        tc.tile_pool(name=f"epsQ{l}", bufs=2, space="PSUM") \
                        as qg_ps, \
                        tc.tile_pool(name=f"epsA{l}", bufs=2, space="PSUM") \
                        as acc_ps:
                    acc_tiles = {}
                    for g in range(nbatch):
                        t0 = g * G
                        gb = min(G, tot_tiles - t0)
                        ne = gb * 128
                        kvg = gbuf.tile([P, G, 2 * D], bf16, tag="kvg",
                                        name="kvg")
                        nc.gpsimd.dma_gather(
                            kvg[:, :gb, :], kv_full[:],
                            idx_src[:, t0 * 8:t0 * 8 + ne // 16],
                            ne, ne, 2 * D)
                        eat = gbuf.tile([ED, G * 128], bf16, tag="eat",
                                        name="eat")
                        nc.sync.dma_start(
                            out=eat[:, :ne],
                            in_=ea_d[:, t0 * 128:t0 * 128 + ne])
                        oh = gbuf.tile([P, G, P], bf16, tag="oh", name="oh")
                        nc.vector.tensor_tensor(
                            out=oh[:, :gb, :],
                            in0=_bcast_mid(rix[:], gb),
                            in1=_bcast3(dcol[:, t0:t0 + gb], P),
                            op=OP.is_equal)
                        ohTs = gbuf.tile([P, G, P], bf16, tag="ohTs",
                                         name="ohTs")
                        nc.sync.dma_start(
                            out=ohTs[:, :gb, :],
                            in_=ohT_d[:, t0 * 128:t0 * 128 + ne])

                        if elvl < 2:
                            continue
                        for bb in range(math.ceil(gb / B)):
                            nb = min(B, gb - bb * B)
                            kvpe = eps_ps.tile([P, B, 2 * D], fp32,
                                               space="PSUM", tag="kvpe",
                                               name="kvpe")
                            qgp = qg_ps.tile([P, B, D], fp32, space="PSUM",
                                             tag="qgp", name="qgp")
                            for u in range(nb):
                                te = bb * B + u
                                tid = t0 + te
                                nc.tensor.matmul(
                                    out=kvpe[:, u, :],
                                    lhsT=eat[:, te * 128:(te + 1) * 128],
                                    rhs=ewd[:], start=True, stop=False,
                                    skip_group_check=True)
                                nc.tensor.matmul(
                                    out=kvpe[:, u, :], lhsT=id16[:],
                                    rhs=kvg[:, te, :], start=False, stop=True,
                                    skip_group_check=True)
                                nc.tensor.matmul(
                                    out=qgp[:, u, :],
                                    lhsT=ohTs[:, te, :],
                                    rhs=q_sb[:, tile_win[tid], :],
                                    start=True, stop=True,
                                    skip_group_check=True)
                            if elvl < 3:
                                continue
                            qgs = ebuf.tile([P, B, D], bf16, tag="qgs",
                                            name="qgs")
                            nc.scalar.copy(out=qgs[:, :nb, :],
                                           in_=qgp[:, :nb, :])
                            qk = ebuf.tile([P, B, D], bf16, tag="qk",
                                           name="qk")
                            nc.vector.tensor_tensor(
                                out=qk[:, :nb, :].rearrange(
                                    "p b (h c) -> p b h c", h=H),
                                in0=qgs[:, :nb, :].rearrange(
                                    "p b (h c) -> p b h c", h=H),
                                in1=kvpe[:, :nb, :D].rearrange(
                                    "p b (h c) -> p b h c", h=H),
                                op=OP.mult)
                            al = ebuf.tile([P, B, H], fp32, tag="al",
                                           name="al")
                            nc.vector.tensor_reduce(
                                out=al[:, :nb, :],
                                in_=qk[:, :nb, :].rearrange(
                                    "p b (h c) -> p b h c", h=H),
                                axis=AX.X, op=OP.add)
                            pk = ebuf.tile([P, B, D + 8], bf16, tag="pk",
                                           name="pk")
                            nc.scalar.activation(
                                out=pk[:, :nb, D:], in_=al[:, :nb, :],
                                func=AF.Exp, scale=1.0 / math.sqrt(C))
                            nc.vector.tensor_tensor(
                                out=pk[:, :nb, :D].rearrange(
                                    "p b (h c) -> p b h c", h=H),
                                in0=kvpe[:, :nb, D:].rearrange(
                                    "p b (h c) -> p b h c", h=H),
                                in1=_bcast4(pk[:, :nb, D:], C),
                                op=OP.mult)
                            if elvl < 4:
                                continue
                            for u in range(nb):
                                tid = t0 + bb * B + u
                                w = tile_win[tid]
                                if win_first[tid]:
                                    acc_tiles[w] = acc_ps.tile(
                                        [P, D + 8], fp32, space="PSUM",
                                        tag="acc", name="acc")
                                nc.tensor.matmul(
                                    out=acc_tiles[w][:],
                                    lhsT=oh[:, bb * B + u, :],
                                    rhs=pk[:, u, :],
                                    start=win_first[tid], stop=win_last[tid],
                                    skip_group_check=True)
                                if win_last[tid]:
                                    ac = acc_tiles.pop(w)
                                    dn = nsm.tile([P, H], fp32, tag="dn",
                                                  name="dn")
                                    nc.vector.tensor_scalar_add(
                                        dn[:], ac[:, D:], 1e-16)
                                    rd = nsm.tile([P, H], fp32, tag="rd",
                                                  name="rd")
                                    nc.vector.reciprocal(out=rd[:], in_=dn[:])
                                    mg = ntmp.tile([P, D], fp32, tag="mg",
                                                   name="mg")
                                    nc.vector.tensor_tensor(
                                        out=mg[:].rearrange(
                                            "p (h c) -> p h c", h=H),
                                        in0=ac[:, :D].rearrange(
                                            "p (h c) -> p h c", h=H),
                                        in1=_bcast3(rd[:], C), op=OP.mult)
                                    nc.vector.tensor_tensor(
                                        out=hc_t[:, w, :], in0=mg[:],
                                        in1=skip_t[:, w, :], op=OP.add)

                # ---- FFN phase (op-major batches) ----
                if stage < 9 and stage < 3:
                    continue
                with tc.tile_pool(name=f"npsB{l}", bufs=2, space="PSUM") \
                        as fps:
                    # half A: idn = gelu(hc @ w1) + h
                    for t in range(NT):
                        tp = fps.tile([P, P], fp32, space="PSUM", tag="tp",
                                      name="tp")
                        nc.tensor.transpose(out=tp[:], in_=hc_t[:, t, :],
                                            identity=id32[:])
                        hcT = ntmp.tile([P, P], bf16, tag="hcT", name="hcT")
                        nc.vector.tensor_copy(out=hcT[:], in_=tp[:])
                        t1p = fps.tile([P, D], fp32, space="PSUM", tag="mm",
                                       name="t1p")
                        nc.tensor.matmul(out=t1p[:], lhsT=hcT[:], rhs=w1[:],
                                         start=True, stop=True)
                        t1g = ntmp.tile([P, D], fp32, tag="t1g", name="t1g")
                        nc.scalar.activation(out=t1g[:], in_=t1p[:],
                                             func=AF.Gelu)
                        nc.vector.tensor_tensor(out=idn_t[:, t, :],
                                                in0=t1g[:],
                                                in1=h_t[:, t, :], op=OP.add)
                    # LN2 batch
                    t2_t = nbig.tile([P, NT, D], fp32, tag="hn_t",
                                     name="t2_t")
                    ln_batch(idn_t, lambda t: t2_t[:, t, :])
                    # half B: h = elu(t2 @ w2) + idn
                    for t in range(NT):
                        tp = fps.tile([P, P], fp32, space="PSUM", tag="tp",
                                      name="tp2")
                        nc.tensor.transpose(out=tp[:], in_=t2_t[:, t, :],
                                            identity=id32[:])
                        t2T = ntmp.tile([P, P], bf16, tag="hcT", name="t2T")
                        nc.vector.tensor_copy(out=t2T[:], in_=tp[:])
                        t3p = fps.tile([P, D], fp32, space="PSUM", tag="mm",
                                       name="t3p")
                        nc.tensor.matmul(out=t3p[:], lhsT=t2T[:], rhs=w2[:],
                                         start=True, stop=True)
                        mn = nsm.tile([P, D], fp32, tag="mn", name="mn2")
                        nc.vector.tensor_scalar_min(mn[:], t3p[:], 0.0)
                        em = nsm.tile([P, D], fp32, tag="em", name="em2")
                        nc.scalar.activation(out=em[:], in_=mn[:], func=AF.Exp)
                        mx = nsm.tile([P, D], fp32, tag="mx", name="mx2")
                        nc.vector.tensor_scalar_max(mx[:], t3p[:], 0.0)
                        t4 = nsm.tile([P, D], fp32, tag="t4", name="t4")
                        nc.vector.scalar_tensor_tensor(
                            out=t4[:], in0=em[:], scalar=-1.0, in1=mx[:],
                            op0=OP.add, op1=OP.add)
                        nc.vector.tensor_tensor(out=h_t[:, t, :], in0=t4[:],
                                                in1=idn_t[:, t, :], op=OP.add)

            # ---------------- output head ----------------
            wl = const.tile([D, 4], fp32, tag="wl")
            nc.sync.dma_start(out=wl[:], in_=wl_d[:, :])
            hl_t = nbig.tile([P, NT, D], fp32, tag="hn_t", name="hl_t")
            ln_batch(h_t, lambda t: hl_t[:, t, :])
            with tc.tile_pool(name="psH", bufs=2, space="PSUM") as psh:
                for t in range(NT):
                    tp = psh.tile([P, P], fp32, space="PSUM", tag="tp",
                                  name="tp")
                    nc.tensor.transpose(out=tp[:], in_=hl_t[:, t, :],
                                        identity=id32[:])
                    hnT = ntmp.tile([P, P], fp32, tag="hlT", name="hlT")
                    nc.scalar.copy(out=hnT[:], in_=tp[:])
                    op_ = psh.tile([P, 4], fp32, space="PSUM", tag="mm",
                                   name="op")
                    nc.tensor.matmul(out=op_[:], lhsT=hnT[:], rhs=wl[:],
                                     start=True, stop=True)
                    ot = ntmp.tile([P, 4], fp32, tag="ot", name="ot")
                    nc.scalar.copy(out=ot[:], in_=op_[:])
                    nc.sync.dma_start(out=out_d[t * P:(t + 1) * P, :],
                                      in_=ot[:])

    nc.compile()
    return nc


def prep_inputs(x, edge_index, edge_attr,
                lin0_w, lin0_b,
                q_w, q_b, k_w, k_b, v_w, v_b, e_w, skip_w, skip_b,
                ln1_g, ln1_b, lins_w, lins_b, ln2_g, ln2_b,
                lins2_w, lins2_b, lnl_g, lnl_b, linl_w, linl_b):
    """Host-side sharding/sorting/folding."""
    x = np.asarray(x, np.float32)
    ei = np.asarray(edge_index, np.int64)
    ea = np.asarray(edge_attr, np.float32)
    src, dst = ei[0], ei[1]
    core = dst // NL
    slot = dst - core * NL

    def fold(W, bias, g, b):
        W = np.asarray(W, np.float64)
        Wf = W * np.asarray(g, np.float64)[None, :]
        cf = np.asarray(bias, np.float64) + W @ np.asarray(b, np.float64)
        return Wf.astype(np.float32), cf.astype(np.float32)

    w4T = np.zeros((L, D, 4 * D), np.float32)
    w1T = np.zeros((L, D, D), np.float32)
    w2T = np.zeros((L, D, D), np.float32)
    ewdT = np.zeros((L, ED, 2 * D), np.float32)
    zero_bias = True
    for l in range(L):
        for j, (W, bias) in enumerate([(q_w[l], q_b[l]), (k_w[l], k_b[l]),
                                       (v_w[l], v_b[l]),
                                       (skip_w[l], skip_b[l])]):
            Wf, cf = fold(W, bias, ln1_g[l], ln1_b[l])
            w4T[l, :, j * D:(j + 1) * D] = Wf.T
            zero_bias &= bool(np.abs(cf).max() == 0)
        w1T[l] = np.asarray(lins_w[l]).T
        zero_bias &= bool(np.abs(np.asarray(lins_b[l])).max() == 0)
        Wf, cf = fold(lins2_w[l], lins2_b[l], ln2_g[l], ln2_b[l])
        w2T[l] = Wf.T
        zero_bias &= bool(np.abs(cf).max() == 0)
        ewT = np.asarray(e_w[l]).T.astype(np.float32)   # [ED, D]
        ewdT[l, :, :D] = ewT
        ewdT[l, :, D:] = ewT
    Wl, cl = fold(linl_w, linl_b, lnl_g, lnl_b)
    wlT = np.zeros((D, 4), np.float32)
    wlT[:, :3] = Wl.T
    zero_bias &= bool(np.abs(cl).max() == 0)
    zero_bias &= bool(np.abs(np.asarray(lin0_b)).max() == 0)

    win = slot // 128
    counts = np.zeros((NCORES, NT), np.int64)
    np.add.at(counts, (core, win), 1)
    tiles_per_window = [max(1, int(math.ceil(counts[:, w].max() / 128)))
                        for w in range(NT)]
    tot_tiles = sum(tiles_per_window)
    tot_e = tot_tiles * 128

    in_maps = []
    order_all = np.lexsort((win, core))
    off = np.searchsorted(core[order_all], np.arange(NCORES + 1))
    kvrow_of = (src // NL) * NLP + (src % NL)
    rowidx = np.tile(np.arange(P, dtype=np.float32), (P, 1)).copy()

    for c in range(NCORES):
        oc = order_all[off[c]:off[c + 1]]
        wc = win[oc]
        woff = np.searchsorted(wc, np.arange(NT + 1))
        src_rows = np.zeros(tot_e, np.int16)
        dst_rel = np.full(tot_e, -1.0, np.float32)   # slot within window
        ea_t = np.zeros((ED, tot_e), np.float32)
        base = 0
        for w in range(NT):
            ew_idx = oc[woff[w]:woff[w + 1]]
            k = len(ew_idx)
            sl = slice(base, base + k)
            src_rows[sl] = kvrow_of[ew_idx].astype(np.int16)
            dst_rel[sl] = (slot[ew_idx] - w * 128).astype(np.float32)
            ea_t[:, sl] = ea[ew_idx].T
            base += tiles_per_window[w] * 128
        assert base == tot_e

        ohT = np.zeros((P, tot_e), np.float32)
        real = dst_rel >= 0
        ohT[dst_rel[real].astype(np.int64), np.nonzero(real)[0]] = 1.0
        dst_col = dst_rel.reshape(tot_tiles, P).T.copy()  # [P, tot_tiles]

        def wrap(a):
            return np.tile(a.reshape(tot_e // 16, 16).T, (8, 1)).copy()

        xs = np.zeros((NLP, F), np.float32)
        xs[:NL] = x[c * NL:(c + 1) * NL]
        in_maps.append({
            "x_shard": xs,
            "idx_src": wrap(src_rows),
            "ohT": ohT,
            "dst_col": dst_col,
            "rowidx": rowidx,
            "ea_t": ea_t,
            "w4T": w4T, "w1T": w1T, "w2T": w2T, "ewdT": ewdT,
            "w0T": np.asarray(lin0_w).T.astype(np.float32),
            "ident": np.eye(P, dtype=np.float32),
            "wlT": wlT,
        })
    return in_maps, tiles_per_window, zero_bias


_CACHE = {}


def kernel(**inputs):
    import ml_dtypes
    in_maps, tiles_per_window, zero_bias = prep_inputs(**inputs)
    for m in in_maps:
        m["ohT"] = m["ohT"].astype(ml_dtypes.bfloat16)
        m["ea_t"] = m["ea_t"].astype(ml_dtypes.bfloat16)
        m["ewdT"] = m["ewdT"].astype(ml_dtypes.bfloat16)
        m["w4T"] = m["w4T"].astype(ml_dtypes.bfloat16)
        m["w1T"] = m["w1T"].astype(ml_dtypes.bfloat16)
        m["w2T"] = m["w2T"].astype(ml_dtypes.bfloat16)

    stage = int(os.environ.get("K_STAGE", "9"))
    elvl = int(os.environ.get("K_EDGE", "4"))
    key = (tuple(tiles_per_window), stage, elvl)
    if key not in _CACHE:
        _CACHE[key] = build(tiles_per_window, zero_bias, stage, elvl)
    nc = _CACHE[key]

    global LAST_RESULT
    res = run_bass_kernel_spmd(
        nc, in_maps, core_ids=list(range(NCORES)),
        trace=bool(os.environ.get("K_TRACE")))
    LAST_RESULT = res
    out = np.zeros((N, 3), np.float32)
    for c in range(NCORES):
        out[c * NL:(c + 1) * NL] = res.results[c]["out"][:NL, :3]
    return out
